# revision 44
# baseline (speedup 1.0000x reference)
"""DigitCapsules routing kernel for 8 Trainium2 NeuronCores.

Math: in the reference, u_hat is an explicit broadcast of u_core over the
capsule axis i, so b stays constant along i in every routing iteration,
softmax over i is exactly uniform (1/K), and the whole 3-iteration routing
collapses (exactly, in floating point too) to:

    v[b, i, :] = squash((1/576) * sum_{r,k} x2[b, r, k] * W[b, r, k, :])

broadcast over i = 0..575, where x2 = x.reshape(B, 8, 576).transpose(0, 2, 1).

Sharding: batch dim B=32 across 8 cores, 4 batches per core (data parallel,
per the hint).  Per core: contract over (r, k)=4608 on TensorE (x columns
stationary, W moving, fp32 PSUM accumulation, 5 r-tiles), take the
k-diagonal of the [8, 128] result via an affine-select mask + grouped DVE
reduction, column-sum the per-batch [8, 16] partials into one [4, 16] PSUM
tile with tiny one-hot matmuls, squash on-chip, and write the i-broadcast
output with 0-stride-source DMAs.

Performance notes:
 - The host packs wx = [W | x2] rows partition-major and pre-converts to
   fp16 ([NB, 128, 680]): halves HBM traffic and runs the matmuls at
   1 cycle/row instead of fp32's 4 (PSUM accumulation stays fp32; measured
   end-to-end relative error ~4e-4 against the fp32 reference).
 - All DMAs use flat 2D access patterns with >=1 KB per-partition runs:
   descriptor-generation time on the issuing sequencer scales with AP row
   count (~2-3 ns/row) and was the dominant cost of earlier versions.
 - Input DMAs (8, partition halves) issue on the SP sequencer while the
   Activation sequencer loads its table; output DMAs (4) issue on the
   Activation sequencer.
 - Output rows are written from a [4, 256] tile holding v 16x, so the
   broadcast DMAs move 1 KB packets ([36, 256] per batch).
 - Bacc (not raw Bass): its compile() splits sync waits into event
   semaphores (TRN2 allows one wait per instruction).
 - tensor_tensor_reduce (custom DVE op) hard-crashes the exec unit on this
   runtime - avoid.
"""

import numpy as np

import concourse.bacc as bacc
import concourse.mybir as mybir
import concourse.tile as tile
from concourse.bass_utils import run_bass_kernel_spmd

N_CORES = 8
B, C, H, W_ = 32, 8, 24, 24
R = H * W_          # 576 routes
RP = 640            # padded routes (5 tiles of 128)
KJ = 128            # fused (k=8, j=16) axis
D = 16
NB = B // N_CORES   # 4 batches per core
NTILE = RP // 128   # 5
WX = KJ + C         # 136 = W row + packed x2 row
FREE = NTILE * WX   # 680 fp16 values per partition
RNORM = 1.0 / float(R)
RNORM2 = RNORM * RNORM

_cached_nc = None
_last_in_maps = None


def _build():
    nc = bacc.Bacc(trn_type="TRN2")
    f32 = mybir.dt.float32
    f16 = mybir.dt.float16

    wx_h = nc.dram_tensor("wx", [NB, 128, FREE], f16, kind="ExternalInput")
    out_h = nc.dram_tensor("out", [NB, R, D], f32, kind="ExternalOutput")

    with tile.TileContext(nc) as tc:
        with (
            tc.tile_pool(name="consts", bufs=1) as consts,
            tc.tile_pool(name="wp", bufs=NB) as wp,
            tc.tile_pool(name="gps", bufs=NB, space="PSUM") as gps,
            tc.tile_pool(name="tps", bufs=1, space="PSUM") as tps,
            tc.tile_pool(name="vps", bufs=1, space="PSUM") as vps,
            tc.tile_pool(name="sm", bufs=24) as sm,
        ):
            # mask[k, k'*16+j] = (k == k'): selects the k-diagonal of G.
            mask_raw = consts.tile([8, KJ], f32)
            nc.gpsimd.memset(mask_raw[:], 1.0)
            nc.gpsimd.affine_select(
                out=mask_raw[:], in_=mask_raw[:],
                compare_op=mybir.AluOpType.is_equal, fill=0.0,
                base=0, pattern=[[1, 8], [0, 16]], channel_multiplier=-1,
            )
            mask_t = consts.tile([8, KJ], f32)
            nc.vector.tensor_copy(mask_t[:], mask_raw[:])
            # oneh[:, n*4+m] = (m == n): column-sums R1[n] into row n of T.
            oneh_t = consts.tile([8, 4 * NB], f32)
            nc.vector.memset(oneh_t[:], 0.0)
            for n in range(NB):
                nc.vector.memset(oneh_t[:, n * 5:n * 5 + 1], 1.0)
            eps_t = consts.tile([NB, 1], f32)
            nc.vector.memset(eps_t[:], 1e-8)
            # sel[n, p] = (p // 32 == n): spreads v across all 128
            # partitions so the output DMA engages all 16 DMA engines
            # (engine = source partition mod 16).
            sel_raw = consts.tile([NB, 128], f32)
            nc.gpsimd.memset(sel_raw[:], 1.0)
            nc.gpsimd.affine_select(
                out=sel_raw[:], in_=sel_raw[:],
                compare_op=mybir.AluOpType.is_ge, fill=0.0,
                base=0, pattern=[[1, 128]], channel_multiplier=-32,
            )
            nc.gpsimd.affine_select(
                out=sel_raw[:], in_=sel_raw[:],
                compare_op=mybir.AluOpType.is_ge, fill=0.0,
                base=31, pattern=[[-1, 128]], channel_multiplier=32,
            )
            sel_t = consts.tile([NB, 128], mybir.dt.float32r)
            nc.vector.tensor_copy(sel_t[:], sel_raw[:])

            # G[n][k, k'*16+j] = sum_r x2[n, r, k] * W[n, r, k'*16+j]
            r1s = []
            for n in range(NB):
                wx_t = wp.tile([128, FREE], f16)
                # One full-tile DMA per batch: a single dma_start already
                # stripes its partition rows over all 16 DMA engines (engine
                # = partition mod 16), and descriptor generation costs
                # ~600 ns per dma_start on the issuing sequencer — fewer is
                # faster.  Batch 0 is split into halves issued first on both
                # sequencers so its completion (which gates the first
                # matmul) comes as early as possible.
                if n == 0:
                    nc.sync.dma_start(wx_t[0:64], wx_h[n, 0:64])
                    nc.scalar.dma_start(wx_t[64:128], wx_h[n, 64:128])
                else:
                    eng = nc.sync if n % 2 == 1 else nc.scalar
                    eng.dma_start(wx_t[:], wx_h[n])
                wx_v = wx_t[:].rearrange("p (d f) -> p d f", f=WX)
                g = gps.tile([8, KJ], f32)
                for d in range(NTILE):
                    nc.tensor.matmul(
                        g[:], wx_v[:, d, KJ:WX], wx_v[:, d, :KJ],
                        start=(d == 0), stop=(d == NTILE - 1),
                    )
                pm = sm.tile([8, KJ], f32)
                nc.vector.tensor_mul(pm[:], g[:], mask_t[:])
                # R1[k, j] = sum_k' pm[k, k'*16+j]  (only k'==k is nonzero)
                r1 = sm.tile([8, D], f32)
                nc.vector.reduce_sum(
                    r1[:], pm[:].rearrange("p (k j) -> p j k", j=D),
                    axis=mybir.AxisListType.X,
                )
                r1s.append(r1)

            # T[n, j] = sum_k R1[n][k, j]  (tiny N=16 one-hot matmuls)
            t_ps = tps.tile([NB, D], f32)
            for n in range(NB):
                nc.tensor.matmul(
                    t_ps[:], oneh_t[:, n * 4:(n + 1) * 4], r1s[n][:],
                    start=(n == 0), stop=(n == NB - 1),
                )

            # squash on T (s = T/576 folded into the scalars):
            #   normT = sum_j T^2;  norm = normT/576^2
            #   v = T * (norm/576) / ((1+norm) * sqrt(norm + 1e-8))
            # square on DVE: scalar.square would evict Sqrt's ACT table and
            # force a ~1.3 us mid-kernel table reload
            t_sb = sm.tile([NB, D], f32)
            nc.vector.tensor_copy(t_sb[:], t_ps[:])
            sq = sm.tile([NB, D], f32)
            nc.vector.tensor_mul(sq[:], t_sb[:], t_sb[:])
            norm_t = sm.tile([NB, 1], f32)
            nc.vector.reduce_sum(norm_t[:], sq[:], axis=mybir.AxisListType.X)
            q = sm.tile([NB, 1], f32)
            nc.scalar.activation(
                q[:], norm_t[:], mybir.ActivationFunctionType.Sqrt,
                bias=eps_t[:], scale=RNORM2,
            )
            a1 = sm.tile([NB, 1], f32)
            nc.vector.tensor_scalar(
                out=a1[:], in0=norm_t[:], scalar1=RNORM2, scalar2=1.0,
                op0=mybir.AluOpType.mult, op1=mybir.AluOpType.add,
            )
            den = sm.tile([NB, 1], f32)
            nc.vector.tensor_mul(den[:], a1[:], q[:])
            rec = sm.tile([NB, 1], f32)
            nc.vector.reciprocal(rec[:], den[:])
            c1 = sm.tile([NB, 1], f32)
            nc.vector.tensor_scalar_mul(c1[:], norm_t[:], RNORM2 * RNORM)

            # one broadcast matmul spreads v to [128, 288] (partition p holds
            # v[p//32] x18) so the single output DMA is striped over all 16
            # DMA engines; the x18 replication is a 0-stride rhs AP.
            # float32r: 1 cycle/row (vs fp32's 4) at N>=288; near-exact here
            # since sel is 0/1 and v passes through unscaled.
            v1 = sm.tile([NB, D], mybir.dt.float32r)
            nc.vector.tensor_scalar(
                out=v1[:], in0=t_ps[:], scalar1=c1[:], scalar2=rec[:],
                op0=mybir.AluOpType.mult, op1=mybir.AluOpType.mult,
            )
            vb_ps = vps.tile([128, 18 * D], f32)
            nc.tensor.matmul(
                vb_ps[:], sel_t[:],
                v1[:].unsqueeze(1).broadcast_to([NB, 18, D]),
                start=True, stop=True)
            vb = sm.tile([128, 18 * D], f32)
            nc.vector.tensor_copy(vb[:], vb_ps[:])
            dst = out_h[:, :, :].flatten().rearrange("(p c) -> p c", c=18 * D)
            nc.sync.dma_start(dst, vb[:])

    nc.finalize()
    return nc


def kernel(x, route_weights):
    global _cached_nc, _last_in_maps
    if _cached_nc is None:
        _cached_nc = _build()
    nc = _cached_nc

    x = np.ascontiguousarray(np.asarray(x), dtype=np.float32)
    w = np.ascontiguousarray(np.asarray(route_weights), dtype=np.float32)
    x2 = x.reshape(B, C, R).transpose(0, 2, 1)          # [B, R, 8]
    wf = w.reshape(B, R, KJ)
    wx = np.zeros((B, RP, WX), np.float32)
    wx[:, :R, :KJ] = wf
    wx[:, :R, KJ:] = x2
    # partition-major tiling, fp16: [B, 128, NTILE*WX]
    wxt = np.ascontiguousarray(
        wx.reshape(B, NTILE, 128, WX).transpose(0, 2, 1, 3)
        .reshape(B, 128, FREE)).astype(np.float16)

    in_maps = [
        {"wx": np.ascontiguousarray(wxt[c * NB:(c + 1) * NB])}
        for c in range(N_CORES)
    ]
    _last_in_maps = in_maps

    res = run_bass_kernel_spmd(nc, in_maps, core_ids=list(range(N_CORES)))
    return np.concatenate([r["out"] for r in res.results], axis=0)


# revision 45
# speedup vs baseline: 1.0443x; 1.0443x over previous
"""DigitCapsules routing kernel for 8 Trainium2 NeuronCores.

Math: in the reference, u_hat is an explicit broadcast of u_core over the
capsule axis i, so b stays constant along i in every routing iteration,
softmax over i is exactly uniform (1/K), and the whole 3-iteration routing
collapses (exactly, in floating point too) to:

    v[b, i, :] = squash((1/576) * sum_{r,k} x2[b, r, k] * W[b, r, k, :])

broadcast over i = 0..575, where x2 = x.reshape(B, 8, 576).transpose(0, 2, 1).

Sharding: batch dim B=32 across 8 cores, 4 batches per core (data parallel,
per the hint).  Per core: contract over (r, k)=4608 on TensorE (x columns
stationary, W moving, fp32 PSUM accumulation, 5 r-tiles), take the
k-diagonal of the [8, 128] result via an affine-select mask + grouped DVE
reduction, column-sum the per-batch [8, 16] partials into one [4, 16] PSUM
tile with tiny one-hot matmuls, squash on-chip, and write the i-broadcast
output with 0-stride-source DMAs.

Performance notes:
 - The host packs wx = [W | x2] rows partition-major and pre-converts to
   fp16 ([NB, 128, 680]): halves HBM traffic and runs the matmuls at
   1 cycle/row instead of fp32's 4 (PSUM accumulation stays fp32; measured
   end-to-end relative error ~4e-4 against the fp32 reference).
 - All DMAs use flat 2D access patterns with >=1 KB per-partition runs:
   descriptor-generation time on the issuing sequencer scales with AP row
   count (~2-3 ns/row) and was the dominant cost of earlier versions.
 - Input DMAs (8, partition halves) issue on the SP sequencer while the
   Activation sequencer loads its table; output DMAs (4) issue on the
   Activation sequencer.
 - Output rows are written from a [4, 256] tile holding v 16x, so the
   broadcast DMAs move 1 KB packets ([36, 256] per batch).
 - Bacc (not raw Bass): its compile() splits sync waits into event
   semaphores (TRN2 allows one wait per instruction).
 - tensor_tensor_reduce (custom DVE op) hard-crashes the exec unit on this
   runtime - avoid.
"""

import numpy as np

import concourse.bacc as bacc
import concourse.mybir as mybir
import concourse.tile as tile
from concourse.bass_utils import run_bass_kernel_spmd

N_CORES = 8
B, C, H, W_ = 32, 8, 24, 24
R = H * W_          # 576 routes
RP = 640            # padded routes (5 tiles of 128)
KJ = 128            # fused (k=8, j=16) axis
D = 16
NB = B // N_CORES   # 4 batches per core
NTILE = RP // 128   # 5
WX = KJ + C         # 136 = W row + packed x2 row
FREE = NTILE * WX   # 680 fp16 values per partition
RNORM = 1.0 / float(R)
RNORM2 = RNORM * RNORM

_cached_nc = None
_last_in_maps = None


def _build():
    nc = bacc.Bacc(trn_type="TRN2")
    f32 = mybir.dt.float32
    f16 = mybir.dt.float16

    wx_h = nc.dram_tensor("wx", [NB, 128, FREE], f16, kind="ExternalInput")
    out_h = nc.dram_tensor("out", [NB, R, D], f32, kind="ExternalOutput")

    with tile.TileContext(nc) as tc:
        with (
            tc.tile_pool(name="consts", bufs=1) as consts,
            tc.tile_pool(name="wp", bufs=NB) as wp,
            tc.tile_pool(name="gps", bufs=NB, space="PSUM") as gps,
            tc.tile_pool(name="tps", bufs=1, space="PSUM") as tps,
            tc.tile_pool(name="vps", bufs=1, space="PSUM") as vps,
            tc.tile_pool(name="sm", bufs=24) as sm,
        ):
            # mask[k, k'*16+j] = (k == k'): selects the k-diagonal of G.
            mask_raw = consts.tile([8, KJ], f32)
            nc.gpsimd.memset(mask_raw[:], 1.0)
            nc.gpsimd.affine_select(
                out=mask_raw[:], in_=mask_raw[:],
                compare_op=mybir.AluOpType.is_equal, fill=0.0,
                base=0, pattern=[[1, 8], [0, 16]], channel_multiplier=-1,
            )
            mask_t = consts.tile([8, KJ], f32)
            nc.vector.tensor_copy(mask_t[:], mask_raw[:])
            # oneh[:, n*4+m] = (m == n): column-sums R1[n] into row n of T.
            oneh_t = consts.tile([8, 4 * NB], f32)
            nc.vector.memset(oneh_t[:], 0.0)
            for n in range(NB):
                nc.vector.memset(oneh_t[:, n * 5:n * 5 + 1], 1.0)
            eps_t = consts.tile([NB, 1], f32)
            nc.vector.memset(eps_t[:], 1e-8)
            # sel[n, p] = (p // 32 == n): spreads v across all 128
            # partitions so the output DMA engages all 16 DMA engines
            # (engine = source partition mod 16).
            sel_raw = consts.tile([NB, 128], f32)
            nc.gpsimd.memset(sel_raw[:], 1.0)
            nc.gpsimd.affine_select(
                out=sel_raw[:], in_=sel_raw[:],
                compare_op=mybir.AluOpType.is_ge, fill=0.0,
                base=0, pattern=[[1, 128]], channel_multiplier=-32,
            )
            nc.gpsimd.affine_select(
                out=sel_raw[:], in_=sel_raw[:],
                compare_op=mybir.AluOpType.is_ge, fill=0.0,
                base=31, pattern=[[-1, 128]], channel_multiplier=32,
            )
            sel_t = consts.tile([NB, 128], mybir.dt.float32r)
            nc.vector.tensor_copy(sel_t[:], sel_raw[:])

            # G[n][k, k'*16+j] = sum_r x2[n, r, k] * W[n, r, k'*16+j]
            r1s = []
            for n in range(NB):
                wx_t = wp.tile([128, FREE], f16)
                # One full-tile DMA per batch: a single dma_start already
                # stripes its partition rows over all 16 DMA engines (engine
                # = partition mod 16), and descriptor generation costs
                # ~600 ns per dma_start on the issuing sequencer — fewer is
                # faster.  Batch 0 is split into halves issued first on both
                # sequencers so its completion (which gates the first
                # matmul) comes as early as possible.
                eng = nc.sync if n % 2 == 0 else nc.scalar
                eng.dma_start(wx_t[:], wx_h[n])
                wx_v = wx_t[:].rearrange("p (d f) -> p d f", f=WX)
                g = gps.tile([8, KJ], f32)
                for d in range(NTILE):
                    nc.tensor.matmul(
                        g[:], wx_v[:, d, KJ:WX], wx_v[:, d, :KJ],
                        start=(d == 0), stop=(d == NTILE - 1),
                    )
                pm = sm.tile([8, KJ], f32)
                nc.vector.tensor_mul(pm[:], g[:], mask_t[:])
                # R1[k, j] = sum_k' pm[k, k'*16+j]  (only k'==k is nonzero)
                r1 = sm.tile([8, D], f32)
                nc.vector.reduce_sum(
                    r1[:], pm[:].rearrange("p (k j) -> p j k", j=D),
                    axis=mybir.AxisListType.X,
                )
                r1s.append(r1)

            # T[n, j] = sum_k R1[n][k, j]  (tiny N=16 one-hot matmuls)
            t_ps = tps.tile([NB, D], f32)
            for n in range(NB):
                nc.tensor.matmul(
                    t_ps[:], oneh_t[:, n * 4:(n + 1) * 4], r1s[n][:],
                    start=(n == 0), stop=(n == NB - 1),
                )

            # squash on T (s = T/576 folded into the scalars):
            #   normT = sum_j T^2;  norm = normT/576^2
            #   v = T * (norm/576) / ((1+norm) * sqrt(norm + 1e-8))
            # square on DVE: scalar.square would evict Sqrt's ACT table and
            # force a ~1.3 us mid-kernel table reload
            t_sb = sm.tile([NB, D], f32)
            nc.vector.tensor_copy(t_sb[:], t_ps[:])
            sq = sm.tile([NB, D], f32)
            nc.vector.tensor_mul(sq[:], t_sb[:], t_sb[:])
            norm_t = sm.tile([NB, 1], f32)
            nc.vector.reduce_sum(norm_t[:], sq[:], axis=mybir.AxisListType.X)
            q = sm.tile([NB, 1], f32)
            nc.scalar.activation(
                q[:], norm_t[:], mybir.ActivationFunctionType.Sqrt,
                bias=eps_t[:], scale=RNORM2,
            )
            a1 = sm.tile([NB, 1], f32)
            nc.vector.tensor_scalar(
                out=a1[:], in0=norm_t[:], scalar1=RNORM2, scalar2=1.0,
                op0=mybir.AluOpType.mult, op1=mybir.AluOpType.add,
            )
            den = sm.tile([NB, 1], f32)
            nc.vector.tensor_mul(den[:], a1[:], q[:])
            rec = sm.tile([NB, 1], f32)
            nc.vector.reciprocal(rec[:], den[:])
            c1 = sm.tile([NB, 1], f32)
            nc.vector.tensor_scalar_mul(c1[:], norm_t[:], RNORM2 * RNORM)

            # one broadcast matmul spreads v to [128, 288] (partition p holds
            # v[p//32] x18) so the single output DMA is striped over all 16
            # DMA engines; the x18 replication is a 0-stride rhs AP.
            # float32r: 1 cycle/row (vs fp32's 4) at N>=288; near-exact here
            # since sel is 0/1 and v passes through unscaled.
            v1 = sm.tile([NB, D], mybir.dt.float32r)
            nc.vector.tensor_scalar(
                out=v1[:], in0=t_ps[:], scalar1=c1[:], scalar2=rec[:],
                op0=mybir.AluOpType.mult, op1=mybir.AluOpType.mult,
            )
            vb_ps = vps.tile([128, 18 * D], f32)
            nc.tensor.matmul(
                vb_ps[:], sel_t[:],
                v1[:].unsqueeze(1).broadcast_to([NB, 18, D]),
                start=True, stop=True)
            vb = sm.tile([128, 18 * D], f32)
            nc.vector.tensor_copy(vb[:], vb_ps[:])
            dst = out_h[:, :, :].flatten().rearrange("(p c) -> p c", c=18 * D)
            nc.sync.dma_start(dst, vb[:])

    nc.finalize()
    return nc


def kernel(x, route_weights):
    global _cached_nc, _last_in_maps
    if _cached_nc is None:
        _cached_nc = _build()
    nc = _cached_nc

    x = np.ascontiguousarray(np.asarray(x), dtype=np.float32)
    w = np.ascontiguousarray(np.asarray(route_weights), dtype=np.float32)
    x2 = x.reshape(B, C, R).transpose(0, 2, 1)          # [B, R, 8]
    wf = w.reshape(B, R, KJ)
    wx = np.zeros((B, RP, WX), np.float32)
    wx[:, :R, :KJ] = wf
    wx[:, :R, KJ:] = x2
    # partition-major tiling, fp16: [B, 128, NTILE*WX]
    wxt = np.ascontiguousarray(
        wx.reshape(B, NTILE, 128, WX).transpose(0, 2, 1, 3)
        .reshape(B, 128, FREE)).astype(np.float16)

    in_maps = [
        {"wx": np.ascontiguousarray(wxt[c * NB:(c + 1) * NB])}
        for c in range(N_CORES)
    ]
    _last_in_maps = in_maps

    res = run_bass_kernel_spmd(nc, in_maps, core_ids=list(range(N_CORES)))
    return np.concatenate([r["out"] for r in res.results], axis=0)
